# revision 3
# baseline (speedup 1.0000x reference)
"""Trainium2 Bass kernel v2 for nn_BinaryClassifier (FFT-frame-mean + 3-layer MLP).

Math identical to v1: layer 1 folds the FFT+mean into W1c = C @ W1.T / 31 so the
device only needs s = sum_f x_f, then a 3-layer MLP.

v2 changes vs the 65us fp16 baseline:
- x ships as fp8 e4m3 (8.125 MB/core, half of fp16) quantized with ERROR
  FEEDBACK across the 31 frames (sigma-delta: q_f = Q(x_f + e_{f-1})), so the
  frame-sum's quantization error is ~1 element's worth instead of sqrt(31)x.
  Simulated end-to-end rel err: 2.3e-3 (vs 2e-2 gate).
- Chunk-major stream: 4 superchunks of 512 features; all 31 frames of a
  superchunk stream consecutively, so each superchunk's sum + its 8 layer-1
  matmuls complete while the next superchunk streams. Tail = last superchunk
  only.
- Frame-sum runs entirely on the PE as identity matmuls in fp8 DoubleRow
  mode: lhsT = [I|I] (two fp8 identities packed, 3D AP [128,2,128]), rhs =
  two consecutive frames viewed [128,2,512] -> one pass accumulates BOTH
  frames into PSUM fp32 at 0.5 cycles/row. 15 pair passes + 1 normal pass
  per superchunk. DVE only does the PSUM->SBUF fp16 copies.
- No gpsimd anywhere (v1 lost ~10us to SWDGE init/drain + the weight-DMA
  crawling on it); weights ride the HWDGE queues, with the 1MB W1c issued
  after superchunk 0's x groups so it lands right before layer 1 needs it.
"""

import os
from contextlib import ExitStack

import numpy as np
import ml_dtypes

import concourse.bacc as bacc
import concourse.bass as bass
import concourse.tile as tile
from concourse import mybir
from concourse.bass_utils import run_bass_kernel_spmd

FRAMES = 31
FFT_LEN = 2048
B = 1024
NCORES = 8
BS = B // NCORES  # 128
P = 128
NSC = 4  # superchunks
SCW = 512  # features per superchunk
NCH = SCW // P  # 4 chunks of 128 features per superchunk
H1 = 256
H2 = 256

F8 = mybir.dt.float8e4
F16 = mybir.dt.float16
F32 = mybir.dt.float32
NP_F8 = mybir.dt.np(F8)  # ml_dtypes.float8_e4m3

# wph (fp16) column layout: W1c chunk-major, then W2T, then W3T
W2TH = (NSC * NCH) * H1  # 16*256 = 4096
W3TH = W2TH + 2 * H2  # 4608
NWH = W3TH + 2  # 4610
# wpk (fp32): b1 (2), b2 (2), b3 (1)
NW = 5

GROUPS = (12, 12, 4, 3)  # frames per DMA within a superchunk

PAIRS = os.environ.get("BASS_V2_PAIRS", "1") == "1"  # DoubleRow on/off


def build_nc(pairs: bool = PAIRS) -> bass.Bass:
    nc = bacc.Bacc("TRN2", debug=False)

    xt_h = nc.dram_tensor("xt", [P, NSC * FRAMES * SCW], F8, kind="ExternalInput")
    wpk_h = nc.dram_tensor("wpk", [P, NW], F32, kind="ExternalInput")
    wph_h = nc.dram_tensor("wph", [P, NWH], F16, kind="ExternalInput")
    wid8_h = nc.dram_tensor("wid8", [P, 2 * P], F8, kind="ExternalInput")
    out_h = nc.dram_tensor("out", [1, BS], F32, kind="ExternalOutput")

    x = xt_h.ap()

    with ExitStack() as ctx:
        tc = ctx.enter_context(tile.TileContext(nc))
        singles = ctx.enter_context(tc.tile_pool(name="singles", bufs=1))
        state = ctx.enter_context(tc.tile_pool(name="state", bufs=1))
        frames_pool = ctx.enter_context(tc.tile_pool(name="frames", bufs=10))
        psum_s = ctx.enter_context(tc.tile_pool(name="psum_s", bufs=2, space="PSUM"))
        ph1 = ctx.enter_context(tc.tile_pool(name="ph1", bufs=1, space="PSUM"))
        pwork = ctx.enter_context(tc.tile_pool(name="pwork", bufs=1, space="PSUM"))

        wpk = singles.tile([P, NW], F32)
        wph = singles.tile([P, NWH], F16)
        wid8 = singles.tile([P, 2 * P], F8)
        scr = state.tile([1, 1], F32, tag="scr")

        # wph DMA'd in 4 slices interleaved at superchunk boundaries so the
        # 1MB weight block never monopolizes a queue mid-stream
        WSLICE = [(0, 1024), (1024, 2048), (2048, 3072), (3072, NWH)]

        wid_pair = wid8.rearrange("p (two n) -> p two n", two=2)

        h1p = [ph1.tile([P, P], F32, tag=f"h1p{m}", name=f"h1p{m}") for m in range(2)]
        h1_sb = state.tile([P, H1], F16, tag="h1_sb")

        gidx = 0
        for s in range(NSC):
            base = s * FRAMES * SCW
            s_ps = psum_s.tile([P, SCW], F32, tag="sps")
            tiles = []
            f0 = 0
            for nf in GROUPS:
                t = frames_pool.tile([P, GROUPS[0] * SCW], F8, tag="xg")
                eng = nc.sync if gidx % 2 == 0 else nc.scalar
                eng.dma_start(
                    out=t[:, : nf * SCW],
                    in_=x[:, base + f0 * SCW : base + (f0 + nf) * SCW],
                )
                gidx += 1
                tiles.append((t, f0, nf))
                f0 += nf
                if s == 0 and gidx == len(GROUPS):
                    # small weights + ACT pre-joins AFTER all of sc0's x
                    # issues: the scalar ENGINE runs dma-issue ops and ACT ops
                    # in one stream, so putting these any earlier delays the
                    # scalar queue's first x transfer by ~1.6us (measured)
                    nc.scalar.dma_start(out=wid8, in_=wid8_h.ap())
                    nc.scalar.dma_start(out=wpk, in_=wpk_h.ap())
                    # PE pre-join on wid8 so hot matmuls keep one data wait
                    dummy_ps = pwork.tile([1, 1], F32, tag="pw")
                    nc.tensor.matmul(
                        dummy_ps, lhsT=wid8[:, 0:1], rhs=wid8[:, 0:1],
                        start=True, stop=True,
                    )
                    # ACT pre-joins: observe wpk; preload the sigmoid table
                    nc.scalar.activation(
                        scr, wpk[0:1, 0:1], mybir.ActivationFunctionType.Copy,
                        bias=0.0, scale=1.0,
                    )
                    nc.scalar.activation(
                        scr, wpk[0:1, 0:1],
                        mybir.ActivationFunctionType.Sigmoid,
                        bias=0.0, scale=1.0,
                    )

            # W1c slice for this superchunk (layer 1 uses it one superchunk
            # later); slice 3 carries the MLP weights too
            c0, c1 = WSLICE[s]
            eng = nc.sync if gidx % 2 == 0 else nc.scalar
            eng.dma_start(out=wph[:, c0:c1], in_=wph_h.ap()[:, c0:c1])
            gidx += 1
            dummy_w = pwork.tile([1, 1], F32, tag="pw")
            nc.tensor.matmul(
                dummy_w, lhsT=wph[:, c0 : c0 + 1], rhs=wph[:, c0 : c0 + 1],
                start=True, stop=True,
            )

            # ---- frame sum on PE (phase-split for the last superchunk so
            # its first 24 frames' sum + L1 run during the stream) ----
            def sum_phase(ph_tiles, ph_ps, last_f):
                npass = 0
                for (t, f0, nf) in ph_tiles:
                    j = 0
                    while j < nf:
                        f = f0 + j
                        if pairs and j + 1 < nf and f + 1 <= last_f:
                            rhs = t[:, j * SCW : (j + 2) * SCW].rearrange(
                                "p (two n) -> p two n", two=2
                            )
                            nc.tensor.matmul(
                                ph_ps,
                                lhsT=wid_pair,
                                rhs=rhs,
                                start=(npass == 0),
                                stop=(f + 2 > last_f),
                                perf_mode=mybir.MatmulPerfMode.DoubleRow,
                            )
                            j += 2
                        else:
                            nc.tensor.matmul(
                                ph_ps,
                                lhsT=wid8[:, 0:P],
                                rhs=t[:, j * SCW : (j + 1) * SCW],
                                start=(npass == 0),
                                stop=(f + 1 > last_f),
                            )
                            j += 1
                        npass += 1

            def copy_and_l1(ph_ps, tagn, l1_stop):
                # PSUM -> SBUF fp16 in 2 halves so layer-1 chunks 0-1 start
                # one half-copy earlier (DVE otherwise idle)
                sT = state.tile([P, SCW], F16, tag=tagn)
                for h in range(2):
                    nc.vector.tensor_copy(
                        sT[:, h * 256 : (h + 1) * 256], ph_ps[:, h * 256 : (h + 1) * 256]
                    )
                for c in range(NCH):
                    kg = s * NCH + c
                    for m in range(2):
                        nc.tensor.matmul(
                            h1p[m],
                            lhsT=wph[:, kg * H1 + m * P : kg * H1 + (m + 1) * P],
                            rhs=sT[:, c * P : (c + 1) * P],
                            start=(kg == 0 and tagn.startswith("st")),
                            stop=(l1_stop and c == NCH - 1),
                        )

            if s < NSC - 1:
                sum_phase(tiles, s_ps, FRAMES - 1)
                copy_and_l1(s_ps, f"st{s}", False)
            else:
                # phase A: frames 0-23 (first two groups)
                sum_phase(tiles[:2], s_ps, 23)
                copy_and_l1(s_ps, f"st{s}", False)
                # phase B: frames 24-30 (last two groups)
                s_ps_b = psum_s.tile([P, SCW], F32, tag="sps")
                sum_phase(tiles[2:], s_ps_b, FRAMES - 1)
                copy_and_l1(s_ps_b, "stb", True)

        # ---- relu 1: m0 on ACT, m1 on DVE (parallel) ----
        nc.scalar.activation(
            h1_sb[:, 0:P],
            h1p[0],
            mybir.ActivationFunctionType.Relu,
            bias=wpk[:, 0:1],
            scale=1.0,
        )
        nc.vector.tensor_scalar(
            h1_sb[:, P : 2 * P], h1p[1], wpk[:, 1:2], 0.0,
            mybir.AluOpType.add, mybir.AluOpType.max,
        )

        # ---- layer 2 ----
        h2_sb = state.tile([P, H2], F16, tag="h2_sb")
        for m in range(2):
            h2p = pwork.tile([P, P], F32, tag="pw2")
            for k in range(2):
                nc.tensor.matmul(
                    h2p,
                    lhsT=wph[:, W2TH + k * H2 + m * P : W2TH + k * H2 + (m + 1) * P],
                    rhs=h1_sb[:, k * P : (k + 1) * P],
                    start=(k == 0),
                    stop=(k == 1),
                )
            nc.scalar.activation(
                h2_sb[:, m * P : (m + 1) * P],
                h2p,
                mybir.ActivationFunctionType.Relu,
                bias=wpk[:, 2 + m : 3 + m],
                scale=1.0,
            )

        # ---- layer 3 + sigmoid ----
        op = pwork.tile([1, P], F32, tag="pw_o")
        for k in range(2):
            nc.tensor.matmul(
                op,
                lhsT=wph[:, W3TH + k : W3TH + k + 1],
                rhs=h2_sb[:, k * P : (k + 1) * P],
                start=(k == 0),
                stop=(k == 1),
            )
        o_sb = state.tile([1, BS], F32, tag="o_sb")
        nc.scalar.activation(
            o_sb,
            op,
            mybir.ActivationFunctionType.Sigmoid,
            bias=wpk[0:1, 4:5],
            scale=1.0,
        )
        nc.sync.dma_start(out=out_h.ap(), in_=o_sb)

    nc.compile()
    return nc


_NC_CACHE: dict = {}


def _get_nc(pairs: bool = PAIRS) -> bass.Bass:
    key = (pairs,)
    if key not in _NC_CACHE:
        _NC_CACHE[key] = build_nc(pairs)
    return _NC_CACHE[key]


_HOST_CACHE: dict = {}


def _host_weights(W1, b1, W2, b2, W3, b3):
    key = (W1.__array_interface__["data"][0], W1.shape)
    if key in _HOST_CACHE:
        return _HOST_CACHE[key]

    n = np.arange(FFT_LEN)
    ang = (2.0 * np.pi / FFT_LEN) * ((n[:, None] * n[None, :]) % FFT_LEN)
    C = np.cos(ang)
    W1c = (C @ W1.astype(np.float64).T / FRAMES).astype(np.float16)  # [2048, 256]
    W2T = W2.astype(np.float16).T  # [256, 256]
    W3T = W3.astype(np.float16).T.reshape(H2)

    wph = np.zeros((P, NWH), dtype=np.float16)
    for kg in range(NSC * NCH):
        wph[:, kg * H1 : (kg + 1) * H1] = W1c[kg * P : (kg + 1) * P, :]
    for k in range(2):
        wph[:, W2TH + k * H2 : W2TH + (k + 1) * H2] = W2T[k * P : (k + 1) * P, :]
    for k in range(2):
        wph[:, W3TH + k] = W3T[k * P : (k + 1) * P]

    wpk = np.zeros((P, NW), dtype=np.float32)
    for m in range(2):
        wpk[:, m] = b1.astype(np.float32)[m * P : (m + 1) * P]
        wpk[:, 2 + m] = b2.astype(np.float32)[m * P : (m + 1) * P]
    wpk[:, 4] = np.float32(b3.reshape(-1)[0])

    eye = np.eye(P, dtype=NP_F8)
    wid8 = np.concatenate([eye, eye], axis=1)  # [128, 256] = [I | I]

    pack = {"wpk": wpk, "wph": wph, "wid8": wid8}
    _HOST_CACHE[key] = pack
    return pack


def _quantize_ef(x: np.ndarray) -> np.ndarray:
    """fp8 e4m3 with error feedback across frames. x: [B, FRAMES*FFT_LEN] f32.
    Returns [B, FRAMES, FFT_LEN] fp8."""
    xf = x.reshape(B, FRAMES, FFT_LEN)
    xq = np.empty((B, FRAMES, FFT_LEN), dtype=NP_F8)
    e = np.zeros((B, FFT_LEN), dtype=np.float32)
    for f in range(FRAMES):
        t = xf[:, f, :] + e
        q = t.astype(NP_F8)
        e = t - q.astype(np.float32)
        xq[:, f, :] = q
    return xq


def _shard_x(x: np.ndarray) -> list:
    """Per-core fp8 tensors: xt[p, s*(31*512) + f*512 + c*128 + b]
    = xq[core*128+b, f, s*512 + c*128 + p]."""
    xq = _quantize_ef(x)
    shards = []
    for core in range(NCORES):
        v = xq[core * BS : (core + 1) * BS]  # [b, f, 2048]
        v = v.reshape(BS, FRAMES, NSC, NCH, P)  # b f s c p
        v = v.transpose(4, 2, 1, 3, 0)  # p s f c b
        shards.append(np.ascontiguousarray(v.reshape(P, NSC * FRAMES * SCW)))
    return shards


def kernel(x, W1, b1, W2, b2, W3, b3, _trace=False, _pairs=None, **_legacy):
    pairs = PAIRS if _pairs is None else _pairs
    x = np.asarray(x, dtype=np.float32)
    pack = _host_weights(
        np.asarray(W1), np.asarray(b1), np.asarray(W2),
        np.asarray(b2), np.asarray(W3), np.asarray(b3),
    )
    in_maps = [{"xt": xt, **pack} for xt in _shard_x(x)]
    nc = _get_nc(pairs)
    res = run_bass_kernel_spmd(
        nc, in_maps, core_ids=list(range(NCORES)), trace=_trace
    )
    out = np.concatenate([res.results[c]["out"][0] for c in range(NCORES)])
    out = out.reshape(B, 1).astype(np.float32)
    if _trace:
        return out, res
    return out
